# revision 26
# baseline (speedup 1.0000x reference)
"""LocalFeatureAggregation Trainium2 kernel (8 NeuronCores, data-parallel over nodes).

v2.1 architecture ("node-major logit table + round-interleaved gather",
on-device one-hot S, gpsimd-hidden accumulation):

  Math identity: with BN folded (g = a*x + c), per-channel softmax over the
  k neighbors depends only on source j = idx[n,k]:
      L[j] = x[j] @ (a*Ws)          (c@Ws shift dropped: softmax-invariant)
      E = exp(L), P = E * x
      feat[n] = a * (sum_k P[j]) / (sum_k E[j]) + c
      out[n]  = (sum_k P / sum_k E) @ (a*Wm) + (c@Wm + bm)

  Phase 1 (replicated per core, 98 strips of 512 nodes, NODE-major):
      x_cm = relu(W1^T-mm + b1)            [PE, DVE evac]
      x_nm-psum = fcm-block-mm @ W1 + ones-row bias mm
      L_nm = x_cm-chunks-mm @ Ws'          [node-major PSUM]
      E = exp(L) -> ep_sb[:, :, 0:256] fp8     [Act evac]
      P = (x_nm max 0) * E -> ep_sb[:, :, 256:512] fp8  [DVE fused]
      one DMA per half-strip writes 256 table rows [E|P] (512B fp8).

  Phase 2 interleaved by ROUNDS over 5 asymmetric subtables
  (6656/14848/14848/8192/5632 rows; int16-safe; 512-aligned): round t's 49
  per-sup gathers overlap subtable t+1 builds; round 4 is a short tail.
  S one-hot is BUILT ON DEVICE (gpsimd is_equal of uint8 dest-lane bytes vs
  an iota row, executed in gather-blocked gaps) instead of 16MB of DMA.
  Segment matmuls (fp8 DoubleRow pairs + odd single) accumulate [sumE|sumP]
  in PSUM; gpsimd copies/adds rounds into an SBUF bf16 accumulator (lagged
  one sup so it hides behind the next gather's ring-block). Round-4 tail
  per sup: sum -> recip(E) -> t=P/E -> SBUF DMA-transpose [scalar] ->
  final matmul (wmp = a*Wm) -> +(c@Wm + bm) -> out.
"""

import os

import numpy as np
import ml_dtypes

import concourse.bass as bass
import concourse.bacc as bacc
import concourse.tile as tile
from concourse import mybir
from concourse.bass_utils import run_bass_kernel_spmd

BN_EPS = 1e-5
P = 128
N_NODES = 50000
K_NBR = 16
C_IN = 128
C2 = 256
C_OUT = 128
N_CORES = 8
NT = 50176                                   # padded table rows (392*128)
TBOUND = (0, 6656, 21504, 36352, 44544, NT)  # 5 subtables, 512-aligned
NSUB = 5
SUP = 128                                    # dest nodes per sup
HS = 256                                     # nodes per phase-1 half-strip
STRIP = 512
GCAP = 1024                                  # cap on idxs per dma_gather
KQUEUES = int(os.environ.get("KQUEUES", "4"))
KSCRATCH = int(os.environ.get("KSCRATCH", "49152"))
GBUFS = int(os.environ.get("KGBUFS", "6"))   # gather tile double-buffering

BF16 = mybir.dt.bfloat16
F32 = mybir.dt.float32
FP8 = mybir.dt.float8e4
U8 = mybir.dt.uint8

_PROG_CACHE = {}


def build_program(nsup, sizes, exact):
    """sizes: [NSUB][nsup] padded slot counts (multiples of 128, <= GCAP).
    exact: [NSUB][nsup] max-over-cores true edge counts (<= sizes)."""
    nodes_pc = nsup * SUP
    nc = bacc.Bacc("TRN2", num_devices=N_CORES, num_swdge_queues=KQUEUES,
                   dynamic_dma_scratch_size=KSCRATCH)

    fcm_d = nc.dram_tensor("fcm", [P, NT], BF16, kind="ExternalInput")
    w1_d = nc.dram_tensor("w1", [C_IN, C2], BF16, kind="ExternalInput")
    ws_d = nc.dram_tensor("wsp", [C2, C2], BF16, kind="ExternalInput")
    wm_d = nc.dram_tensor("wmp", [C2, C_OUT], BF16, kind="ExternalInput")
    b1_d = nc.dram_tensor("b1c", [P, 2], F32, kind="ExternalInput")
    b1r_d = nc.dram_tensor("b1r", [1, C2], BF16, kind="ExternalInput")
    bm_d = nc.dram_tensor("bmr", [1, C_OUT], F32, kind="ExternalInput")
    iot_d = nc.dram_tensor("iot", [1, P], U8, kind="ExternalInput")

    tot_chunks = sum(sizes[t][s] for t in range(NSUB) for s in range(nsup)) // P
    idx_tot = tot_chunks * P
    idx_d = nc.dram_tensor("idxs", [P, idx_tot // 16], mybir.dt.int16,
                           kind="ExternalInput")
    nloc_d = nc.dram_tensor("nlocs", [P, tot_chunks], U8, kind="ExternalInput")

    tab_d = nc.dram_tensor("tab", [NT, 2 * C2], FP8, kind="Internal")
    out_d = nc.dram_tensor("out", [nodes_pc, C_OUT], F32, kind="ExternalOutput")

    n_strips = [(TBOUND[t + 1] - TBOUND[t]) // STRIP for t in range(NSUB)]

    with tile.TileContext(nc) as tc:
        with (
            tc.tile_pool(name="consts", bufs=1) as consts,
            tc.tile_pool(name="fpool", bufs=2) as fpool,
            tc.tile_pool(name="p1w", bufs=2) as p1w,
            tc.tile_pool(name="gpool", bufs=GBUFS) as gpool,
            tc.tile_pool(name="spool", bufs=3) as spool,
            tc.tile_pool(name="p2w", bufs=2) as p2w,
            tc.tile_pool(name="tpool", bufs=1) as tpool,
            tc.tile_pool(name="opool", bufs=3) as opool,
            tc.tile_pool(name="xps", bufs=1, space="PSUM") as xps_p,
            tc.tile_pool(name="xnm", bufs=2, space="PSUM") as xnm_p,
            tc.tile_pool(name="lps", bufs=2, space="PSUM") as lps_p,
            tc.tile_pool(name="accp", bufs=2, space="PSUM") as acc_p,
            tc.tile_pool(name="opsp", bufs=1, space="PSUM") as ops_p,
        ):
            # ---- constants -----------------------------------------------
            w1s = consts.tile([P, C2], BF16)
            nc.sync.dma_start(out=w1s, in_=w1_d[:, :])
            wss = consts.tile([P, 2, C2], BF16)       # [k-chunk][ch', free]
            for k in range(2):
                nc.sync.dma_start(out=wss[:, k, :], in_=ws_d[k * P:(k + 1) * P, :])
            wms = consts.tile([P, 2, C_OUT], BF16)
            for k in range(2):
                nc.sync.dma_start(out=wms[:, k, :], in_=wm_d[k * P:(k + 1) * P, :])
            b1c = consts.tile([P, 2], F32)
            nc.sync.dma_start(out=b1c, in_=b1_d[:, :])
            b1r = consts.tile([1, C2], BF16)
            nc.sync.dma_start(out=b1r, in_=b1r_d[:, :])
            ones = consts.tile([1, P], BF16)
            nc.vector.memset(ones, 1.0)
            bmb = consts.tile([P, C_OUT], F32)
            nc.sync.dma_start(
                out=bmb, in_=bass.AP(tensor=bm_d, offset=0, ap=[[0, P], [1, C_OUT]])
            )
            iota = consts.tile([P, P], U8)
            nc.sync.dma_start(
                out=iota, in_=bass.AP(tensor=iot_d, offset=0, ap=[[0, P], [1, P]])
            )
            idx_all = tpool.tile([P, idx_tot // 16], mybir.dt.int16, tag="idx")
            nc.sync.dma_start(out=idx_all, in_=idx_d[:, :])
            nloc_sb = tpool.tile([P, tot_chunks], U8, tag="nloc")
            nc.scalar.dma_start(out=nloc_sb, in_=nloc_d[:, :])
            acc_sb = tpool.tile([P, nsup, 2 * C2], BF16, tag="acc")
            # pre-zero the gather arena (all GBUFS rotating buffers) so that
            # exact-count gathers never leave NaN-able stale bytes for the
            # S=0 columns to multiply.
            maxnch = max(sizes[t][s] for t in range(NSUB) for s in range(nsup)) // P
            for _ in range(GBUFS):
                gz = gpool.tile([P, maxnch, 2 * C2], FP8, tag="g")
                nc.vector.memset(gz, 0.0)

            # ---- phase-1 strip build (512 nodes, two half-strips) --------
            def build_strip(nr0):
                fstrip = fpool.tile([P, STRIP], BF16)
                nc.sync.dma_start(out=fstrip, in_=fcm_d[:, nr0:nr0 + STRIP])
                for half in range(2):
                    nr = nr0 + half * HS
                    fs = fstrip[:, half * HS:(half + 1) * HS]
                    x_ps = xps_p.tile([P, 2, HS], F32, tag="xps")
                    for h in range(2):
                        nc.tensor.matmul(
                            x_ps[:, h, :], lhsT=w1s[:, h * P:(h + 1) * P],
                            rhs=fs, start=True, stop=True,
                        )
                    x_sb = p1w.tile([P, 2, HS], BF16, tag="x")
                    for h in range(2):
                        nc.vector.tensor_scalar(
                            out=x_sb[:, h, :], in0=x_ps[:, h, :],
                            scalar1=b1c[:, h:h + 1], scalar2=0.0,
                            op0=mybir.AluOpType.add, op1=mybir.AluOpType.max,
                        )
                    xn_ps = xnm_p.tile([P, 2, C2], F32, tag="xnm")
                    for b in range(2):
                        nc.tensor.matmul(
                            xn_ps[:, b, :], lhsT=fs[:, b * P:(b + 1) * P],
                            rhs=w1s[:, :], start=True, stop=False,
                        )
                        nc.tensor.matmul(
                            xn_ps[:, b, :], lhsT=ones, rhs=b1r,
                            start=False, stop=True,
                        )
                    l_ps = lps_p.tile([P, 2, C2], F32, tag="lps")
                    for b in range(2):
                        for h in range(2):
                            nc.tensor.matmul(
                                l_ps[:, b, :],
                                lhsT=x_sb[:, h, b * P:(b + 1) * P],
                                rhs=wss[:, h, :],
                                start=(h == 0), stop=(h == 1),
                            )
                    ep_sb = p1w.tile([P, 2, 2 * C2], FP8, tag="ep")
                    nc.scalar.activation(
                        out=ep_sb[:, :, 0:C2], in_=l_ps,
                        func=mybir.ActivationFunctionType.Exp, scale=1.0,
                    )
                    nc.vector.scalar_tensor_tensor(
                        out=ep_sb[:, :, C2:2 * C2], in0=xn_ps, scalar=0.0,
                        in1=ep_sb[:, :, 0:C2],
                        op0=mybir.AluOpType.max, op1=mybir.AluOpType.mult,
                    )
                    nc.sync.dma_start(
                        out=bass.AP(tensor=tab_d, offset=nr * 2 * C2,
                                    ap=[[2 * C2, P], [P * 2 * C2, 2], [1, 2 * C2]]),
                        in_=ep_sb,
                    )

            # ---- per-sup round processing --------------------------------
            idx_off = [0]
            chunk_off = [0]
            TB = 8
            batch = {}
            def acc_fold(t, s, acc):
                if t == 0:
                    nc.vector.tensor_copy(acc_sb[:, s, :], acc)
                else:
                    nc.vector.tensor_tensor(
                        out=acc_sb[:, s, :], in0=acc_sb[:, s, :], in1=acc,
                        op=mybir.AluOpType.add,
                    )

            def process_sup(t, s):
                ssz = sizes[t][s]
                nch = ssz // P
                nex = exact[t][s]
                nchu = -(-nex // P)          # chunks actually carrying edges
                # on-device one-hot S: S[p, c, d] = (nloc[p, c] == d)
                s_t = spool.tile([P, nchu, P], FP8, tag="s")
                co = chunk_off[0]
                nl_ap = bass.AP(
                    tensor=nloc_sb.tensor,
                    offset=nloc_sb[:, co:co + nchu].offset,
                    ap=[nloc_sb[:, co:co + nchu].ap[0],
                        nloc_sb[:, co:co + nchu].ap[1], [0, P]],
                )
                io_ap = bass.AP(
                    tensor=iota.tensor, offset=iota.offset,
                    ap=[iota.ap[0], [0, nchu], iota.ap[1]],
                )
                nc.vector.tensor_tensor(
                    out=s_t, in0=nl_ap, in1=io_ap, op=mybir.AluOpType.is_equal,
                )
                g_t = gpool.tile([P, nch, 2 * C2], FP8, tag="g")
                nc.gpsimd.dma_gather(
                    g_t, tab_d[TBOUND[t]:TBOUND[t + 1], :],
                    idx_all[:, idx_off[0]:idx_off[0] + ssz // 16],
                    num_idxs=ssz, num_idxs_reg=nex,
                    elem_size=2 * C2, transpose=False,
                    queue_num=(t * nsup + s) % KQUEUES,
                )
                idx_off[0] += ssz // 16
                chunk_off[0] += nch

                acc = acc_p.tile([P, 2 * C2], F32, tag="acc")
                npair = nchu // 2
                for ci in range(npair):
                    nc.tensor.matmul(
                        acc, lhsT=s_t[:, 2 * ci:2 * ci + 2, :],
                        rhs=g_t[:, 2 * ci:2 * ci + 2, :],
                        start=(ci == 0), stop=(ci == npair - 1 and nchu % 2 == 0),
                        perf_mode=mybir.MatmulPerfMode.DoubleRow,
                    )
                if nchu % 2 == 1:
                    nc.tensor.matmul(
                        acc, lhsT=s_t[:, nchu - 1, :], rhs=g_t[:, nchu - 1, :],
                        start=(nchu == 1), stop=True,
                    )
                if t < NSUB - 1:
                    acc_fold(t, s, acc)
                else:
                    bi = s % TB
                    if bi == 0:
                        sum_tile = p2w.tile([P, TB, 2 * C2], F32, tag="sum")
                        t_tile = p2w.tile([P, TB, C2], BF16, tag="t")
                        batch["sum"], batch["t"] = sum_tile, t_tile
                    sum_b, t_b = batch["sum"], batch["t"]
                    nc.vector.tensor_tensor(
                        out=sum_b[:, bi, :], in0=acc_sb[:, s, :],
                        in1=acc, op=mybir.AluOpType.add,
                    )
                    rec = p2w.tile([P, C2], F32, tag="rec")
                    nc.vector.reciprocal_approx_fast(rec, sum_b[:, bi, 0:C2])
                    nc.vector.tensor_tensor(
                        out=t_b[:, bi, :], in0=sum_b[:, bi, C2:2 * C2],
                        in1=rec, op=mybir.AluOpType.mult,
                    )
                    if bi == TB - 1 or s == nsup - 1:
                        nb_ = bi + 1
                        s0 = s - bi
                        tT_b = p2w.tile([P, 2 * TB, P], BF16, tag="tT")
                        nc.scalar.dma_start(
                            out=tT_b[:, 0:2 * nb_, :], in_=t_b[:, 0:nb_, :],
                            transpose=True,
                        )
                        oo = opool.tile([P, TB, C_OUT], F32)
                        for si in range(nb_):
                            o_ps = ops_p.tile([P, C_OUT], F32, tag="o")
                            for k in range(2):
                                nc.tensor.matmul(
                                    o_ps, lhsT=tT_b[:, 2 * si + k, :],
                                    rhs=wms[:, k, :],
                                    start=(k == 0), stop=(k == 1),
                                )
                            nc.vector.tensor_tensor(
                                out=oo[:, si, :], in0=o_ps, in1=bmb,
                                op=mybir.AluOpType.add,
                            )
                        nc.scalar.dma_start(
                            out=bass.AP(
                                tensor=out_d, offset=s0 * SUP * C_OUT,
                                ap=[[C_OUT, P], [SUP * C_OUT, nb_], [1, C_OUT]],
                            ),
                            in_=oo[:, 0:nb_, :],
                        )

            # ---- interleaved schedule ------------------------------------
            for t in range(NSUB):
                done = 0
                for i in range(n_strips[t]):
                    build_strip(TBOUND[t] + i * STRIP)
                    if t > 0:
                        goal = min(nsup, (nsup * (i + 1)) // n_strips[t])
                        while done < goal:
                            process_sup(t - 1, done)
                            done += 1
                if t > 0:
                    while done < nsup:
                        process_sup(t - 1, done)
                        done += 1
            # interleave the two post-build rounds (r3 remainder handled
            # above; here r4 lags r3 by the build loop) — r4 sup s needs
            # acc_sb[s] final, i.e. r3 sup s done, which the program order
            # guarantees.
            for s in range(nsup):
                process_sup(NSUB - 1, s)

    nc.compile()
    return nc


def _get_prog(nsup, sizes, exact):
    key = (nsup, tuple(map(tuple, sizes)), tuple(map(tuple, exact)),
           KQUEUES, KSCRATCH, GBUFS)
    if key not in _PROG_CACHE:
        _PROG_CACHE[key] = build_program(nsup, sizes, exact)
    return _PROG_CACHE[key]


def _wrap16(flat):
    """int16 index layout for dma_gather: [16-partition wrap, replicated x8]."""
    n = flat.shape[0]
    w = flat.reshape(n // 16, 16).T              # [16, n/16]
    return np.tile(w, (8, 1))                    # [128, n/16]


def prep_inputs(features, neighbor_idx, W1, b1, gamma, beta, run_mean, run_var,
                Ws, Wm, bm, n_cores=N_CORES):
    bf16 = ml_dtypes.bfloat16

    a = (gamma / np.sqrt(run_var + BN_EPS)).astype(np.float32)
    c = (beta - run_mean * a).astype(np.float32)
    wsp = (a[:, None] * Ws).astype(bf16)
    wmp = (a[:, None] * Wm).astype(bf16)
    bmv = (c.astype(np.float64) @ np.asarray(Wm, np.float64) + bm).astype(np.float32)

    fcm = np.zeros((P, NT), dtype=bf16)
    fcm[:, :N_NODES] = np.asarray(features, np.float32).T.astype(bf16)

    n_pc = neighbor_idx.shape[0] // n_cores
    nodes_pc = -(-n_pc // SUP) * SUP
    nsup = nodes_pc // SUP

    ni_all = np.asarray(neighbor_idx, np.int64)
    # core_edges[core][sup][sub] = (j_rel sorted, n_local)
    core_edges = []
    cnt = np.zeros((NSUB, n_cores, nsup), np.int64)
    for ci in range(n_cores):
        ni = ni_all[ci * n_pc:(ci + 1) * n_pc]
        if nodes_pc != n_pc:
            ni = np.concatenate(
                [ni, np.zeros((nodes_pc - n_pc, K_NBR), np.int64)], axis=0)
        per_sup = []
        for s in range(nsup):
            e = ni[s * SUP:(s + 1) * SUP]
            nloc = np.repeat(np.arange(SUP), K_NBR)
            j = e.reshape(-1)
            subs = []
            for t in range(NSUB):
                m = (j >= TBOUND[t]) & (j < TBOUND[t + 1])
                jt, nt_ = j[m] - TBOUND[t], nloc[m]
                o = np.argsort(jt, kind="stable")   # row-sorted for HBM locality
                subs.append((jt[o], nt_[o]))
                cnt[t, ci, s] = m.sum()
            per_sup.append(subs)
        core_edges.append(per_sup)

    sizes = [
        [int(-(-cnt[t, :, s].max() // P) * P) for s in range(nsup)]
        for t in range(NSUB)
    ]
    exact = [
        [int(cnt[t, :, s].max()) for s in range(nsup)]
        for t in range(NSUB)
    ]
    for t in range(NSUB):
        for s in range(nsup):
            assert 0 < sizes[t][s] <= GCAP, (t, s, sizes[t][s])

    tot_chunks = sum(sizes[t][s] for t in range(NSUB) for s in range(nsup)) // P
    idx_tot = tot_chunks * P

    shared = dict(
        fcm=fcm, w1=np.ascontiguousarray(W1.astype(bf16)),
        wsp=np.ascontiguousarray(wsp), wmp=np.ascontiguousarray(wmp),
        b1c=np.ascontiguousarray(b1.astype(np.float32).reshape(2, P).T),
        b1r=np.ascontiguousarray(b1.astype(bf16).reshape(1, C2)),
        bmr=bmv.reshape(1, C_OUT),
        iot=np.arange(P, dtype=np.uint8).reshape(1, P),
    )

    in_maps = []
    for ci in range(n_cores):
        idx_cols = []
        nloc_cols = []
        for t in range(NSUB):
            for s in range(nsup):
                jl, nl = core_edges[ci][s][t]
                size = sizes[t][s]
                idx = np.zeros(size, np.int16)
                idx[:len(jl)] = jl.astype(np.int16)
                idx_cols.append(idx)
                nlc = np.full(size, 255, np.uint8)
                nlc[:len(nl)] = nl.astype(np.uint8)
                nloc_cols.append(nlc)
        idx_flat = np.concatenate(idx_cols)
        assert idx_flat.shape[0] == idx_tot
        idxs = np.ascontiguousarray(_wrap16(idx_flat))
        # nloc layout: value for (chunk c, lane p) at [p, c]
        nlocs = np.ascontiguousarray(
            np.concatenate(nloc_cols).reshape(tot_chunks, P).T)
        in_maps.append(dict(shared, idxs=idxs, nlocs=nlocs))

    return in_maps, nsup, sizes, exact, n_pc


def kernel(**inputs):
    in_maps, nsup, sizes, exact, n_pc = prep_inputs(**inputs)
    nc = _get_prog(nsup, sizes, exact)
    res = run_bass_kernel_spmd(nc, in_maps, core_ids=list(range(N_CORES)))
    return np.concatenate([r["out"][:n_pc] for r in res.results], axis=0)


# revision 27
# speedup vs baseline: 1.0640x; 1.0640x over previous
"""LocalFeatureAggregation Trainium2 kernel (8 NeuronCores, data-parallel over nodes).

v2.1 architecture ("node-major logit table + round-interleaved gather",
on-device one-hot S, gpsimd-hidden accumulation):

  Math identity: with BN folded (g = a*x + c), per-channel softmax over the
  k neighbors depends only on source j = idx[n,k]:
      L[j] = x[j] @ (a*Ws)          (c@Ws shift dropped: softmax-invariant)
      E = exp(L), P = E * x
      feat[n] = a * (sum_k P[j]) / (sum_k E[j]) + c
      out[n]  = (sum_k P / sum_k E) @ (a*Wm) + (c@Wm + bm)

  Phase 1 (replicated per core, 98 strips of 512 nodes, NODE-major):
      x_cm = relu(W1^T-mm + b1)            [PE, DVE evac]
      x_nm-psum = fcm-block-mm @ W1 + ones-row bias mm
      L_nm = x_cm-chunks-mm @ Ws'          [node-major PSUM]
      E = exp(L) -> ep_sb[:, :, 0:256] fp8     [Act evac]
      P = (x_nm max 0) * E -> ep_sb[:, :, 256:512] fp8  [DVE fused]
      one DMA per half-strip writes 256 table rows [E|P] (512B fp8).

  Phase 2 interleaved by ROUNDS over 5 asymmetric subtables
  (6656/14848/14848/8192/5632 rows; int16-safe; 512-aligned): round t's 49
  per-sup gathers overlap subtable t+1 builds; round 4 is a short tail.
  S one-hot is BUILT ON DEVICE (gpsimd is_equal of uint8 dest-lane bytes vs
  an iota row, executed in gather-blocked gaps) instead of 16MB of DMA.
  Segment matmuls (fp8 DoubleRow pairs + odd single) accumulate [sumE|sumP]
  in PSUM; gpsimd copies/adds rounds into an SBUF bf16 accumulator (lagged
  one sup so it hides behind the next gather's ring-block). Round-4 tail
  per sup: sum -> recip(E) -> t=P/E -> SBUF DMA-transpose [scalar] ->
  final matmul (wmp = a*Wm) -> +(c@Wm + bm) -> out.
"""

import os

import numpy as np
import ml_dtypes

import concourse.bass as bass
import concourse.bacc as bacc
import concourse.tile as tile
from concourse import mybir
from concourse.bass_utils import run_bass_kernel_spmd

BN_EPS = 1e-5
P = 128
N_NODES = 50000
K_NBR = 16
C_IN = 128
C2 = 256
C_OUT = 128
N_CORES = 8
NT = 50176                                   # padded table rows (392*128)
TBOUND = (0, 6656, 21504, 36352, 44544, NT)  # 5 subtables, 512-aligned
NSUB = 5
SUP = 128                                    # dest nodes per sup
HS = 256                                     # nodes per phase-1 half-strip
STRIP = 512
GCAP = 1024                                  # cap on idxs per dma_gather
KQUEUES = int(os.environ.get("KQUEUES", "4"))
KSCRATCH = int(os.environ.get("KSCRATCH", "49152"))
GBUFS = int(os.environ.get("KGBUFS", "6"))   # gather tile double-buffering

BF16 = mybir.dt.bfloat16
F32 = mybir.dt.float32
FP8 = mybir.dt.float8e4
U8 = mybir.dt.uint8

_PROG_CACHE = {}


def build_program(nsup, sizes, exact):
    """sizes: [NSUB][nsup] padded slot counts (multiples of 128, <= GCAP).
    exact: [NSUB][nsup] max-over-cores true edge counts (<= sizes)."""
    nodes_pc = nsup * SUP
    nc = bacc.Bacc("TRN2", num_devices=N_CORES, num_swdge_queues=KQUEUES,
                   dynamic_dma_scratch_size=KSCRATCH)

    fcm_d = nc.dram_tensor("fcm", [P, NT], BF16, kind="ExternalInput")
    w1_d = nc.dram_tensor("w1", [C_IN, C2], BF16, kind="ExternalInput")
    ws_d = nc.dram_tensor("wsp", [C2, C2], BF16, kind="ExternalInput")
    wm_d = nc.dram_tensor("wmp", [C2, C_OUT], BF16, kind="ExternalInput")
    b1_d = nc.dram_tensor("b1c", [P, 2], F32, kind="ExternalInput")
    b1r_d = nc.dram_tensor("b1r", [1, C2], BF16, kind="ExternalInput")
    bm_d = nc.dram_tensor("bmr", [1, C_OUT], F32, kind="ExternalInput")
    iot_d = nc.dram_tensor("iot", [1, P], U8, kind="ExternalInput")

    tot_chunks = sum(sizes[t][s] for t in range(NSUB) for s in range(nsup)) // P
    idx_tot = tot_chunks * P
    idx_d = nc.dram_tensor("idxs", [P, idx_tot // 16], mybir.dt.int16,
                           kind="ExternalInput")
    nloc_d = nc.dram_tensor("nlocs", [P, tot_chunks], U8, kind="ExternalInput")

    tab_d = nc.dram_tensor("tab", [NT, 2 * C2], FP8, kind="Internal")
    out_d = nc.dram_tensor("out", [nodes_pc, C_OUT], F32, kind="ExternalOutput")

    n_strips = [(TBOUND[t + 1] - TBOUND[t]) // STRIP for t in range(NSUB)]

    with tile.TileContext(nc) as tc:
        with (
            tc.tile_pool(name="consts", bufs=1) as consts,
            tc.tile_pool(name="fpool", bufs=2) as fpool,
            tc.tile_pool(name="p1w", bufs=3) as p1w,
            tc.tile_pool(name="gpool", bufs=GBUFS) as gpool,
            tc.tile_pool(name="spool", bufs=4) as spool,
            tc.tile_pool(name="p2w", bufs=2) as p2w,
            tc.tile_pool(name="tpool", bufs=1) as tpool,
            tc.tile_pool(name="opool", bufs=3) as opool,
            tc.tile_pool(name="xps", bufs=1, space="PSUM") as xps_p,
            tc.tile_pool(name="xnm", bufs=2, space="PSUM") as xnm_p,
            tc.tile_pool(name="lps", bufs=2, space="PSUM") as lps_p,
            tc.tile_pool(name="accp", bufs=2, space="PSUM") as acc_p,
            tc.tile_pool(name="opsp", bufs=1, space="PSUM") as ops_p,
        ):
            # ---- constants -----------------------------------------------
            w1s = consts.tile([P, C2], BF16)
            nc.sync.dma_start(out=w1s, in_=w1_d[:, :])
            wss = consts.tile([P, 2, C2], BF16)       # [k-chunk][ch', free]
            for k in range(2):
                nc.sync.dma_start(out=wss[:, k, :], in_=ws_d[k * P:(k + 1) * P, :])
            wms = consts.tile([P, 2, C_OUT], BF16)
            for k in range(2):
                nc.sync.dma_start(out=wms[:, k, :], in_=wm_d[k * P:(k + 1) * P, :])
            b1c = consts.tile([P, 2], F32)
            nc.sync.dma_start(out=b1c, in_=b1_d[:, :])
            b1r = consts.tile([1, C2], BF16)
            nc.sync.dma_start(out=b1r, in_=b1r_d[:, :])
            ones = consts.tile([1, P], BF16)
            nc.vector.memset(ones, 1.0)
            bmb = consts.tile([P, C_OUT], F32)
            nc.sync.dma_start(
                out=bmb, in_=bass.AP(tensor=bm_d, offset=0, ap=[[0, P], [1, C_OUT]])
            )
            iota = consts.tile([P, P], U8)
            nc.sync.dma_start(
                out=iota, in_=bass.AP(tensor=iot_d, offset=0, ap=[[0, P], [1, P]])
            )
            idx_all = tpool.tile([P, idx_tot // 16], mybir.dt.int16, tag="idx")
            nc.sync.dma_start(out=idx_all, in_=idx_d[:, :])
            nloc_sb = tpool.tile([P, tot_chunks], U8, tag="nloc")
            nc.scalar.dma_start(out=nloc_sb, in_=nloc_d[:, :])
            acc_sb = tpool.tile([P, nsup, 2 * C2], BF16, tag="acc")
            # pre-zero the gather arena (all GBUFS rotating buffers) so that
            # exact-count gathers never leave NaN-able stale bytes for the
            # S=0 columns to multiply.
            maxnch = max(sizes[t][s] for t in range(NSUB) for s in range(nsup)) // P
            for _ in range(GBUFS):
                gz = gpool.tile([P, maxnch, 2 * C2], FP8, tag="g")
                nc.vector.memset(gz, 0.0)

            # ---- phase-1 strip build (512 nodes, two half-strips) --------
            def build_strip(nr0):
                fstrip = fpool.tile([P, STRIP], BF16)
                nc.sync.dma_start(out=fstrip, in_=fcm_d[:, nr0:nr0 + STRIP])
                for half in range(2):
                    nr = nr0 + half * HS
                    fs = fstrip[:, half * HS:(half + 1) * HS]
                    x_ps = xps_p.tile([P, 2, HS], F32, tag="xps")
                    for h in range(2):
                        nc.tensor.matmul(
                            x_ps[:, h, :], lhsT=w1s[:, h * P:(h + 1) * P],
                            rhs=fs, start=True, stop=True,
                        )
                    x_sb = p1w.tile([P, 2, HS], BF16, tag="x")
                    for h in range(2):
                        nc.vector.tensor_scalar(
                            out=x_sb[:, h, :], in0=x_ps[:, h, :],
                            scalar1=b1c[:, h:h + 1], scalar2=0.0,
                            op0=mybir.AluOpType.add, op1=mybir.AluOpType.max,
                        )
                    xn_ps = xnm_p.tile([P, 2, C2], F32, tag="xnm")
                    for b in range(2):
                        nc.tensor.matmul(
                            xn_ps[:, b, :], lhsT=fs[:, b * P:(b + 1) * P],
                            rhs=w1s[:, :], start=True, stop=False,
                        )
                        nc.tensor.matmul(
                            xn_ps[:, b, :], lhsT=ones, rhs=b1r,
                            start=False, stop=True,
                        )
                    l_ps = lps_p.tile([P, 2, C2], F32, tag="lps")
                    for b in range(2):
                        for h in range(2):
                            nc.tensor.matmul(
                                l_ps[:, b, :],
                                lhsT=x_sb[:, h, b * P:(b + 1) * P],
                                rhs=wss[:, h, :],
                                start=(h == 0), stop=(h == 1),
                            )
                    ep_sb = p1w.tile([P, 2, 2 * C2], FP8, tag="ep")
                    nc.scalar.activation(
                        out=ep_sb[:, :, 0:C2], in_=l_ps,
                        func=mybir.ActivationFunctionType.Exp, scale=1.0,
                    )
                    nc.vector.scalar_tensor_tensor(
                        out=ep_sb[:, :, C2:2 * C2], in0=xn_ps, scalar=0.0,
                        in1=ep_sb[:, :, 0:C2],
                        op0=mybir.AluOpType.max, op1=mybir.AluOpType.mult,
                    )
                    nc.sync.dma_start(
                        out=bass.AP(tensor=tab_d, offset=nr * 2 * C2,
                                    ap=[[2 * C2, P], [P * 2 * C2, 2], [1, 2 * C2]]),
                        in_=ep_sb,
                    )

            # ---- per-sup round processing --------------------------------
            idx_off = [0]
            chunk_off = [0]
            TB = 8
            batch = {}
            def acc_fold(t, s, acc):
                if t == 0:
                    nc.vector.tensor_copy(acc_sb[:, s, :], acc)
                else:
                    nc.vector.tensor_tensor(
                        out=acc_sb[:, s, :], in0=acc_sb[:, s, :], in1=acc,
                        op=mybir.AluOpType.add,
                    )

            def process_sup(t, s):
                ssz = sizes[t][s]
                nch = ssz // P
                nex = exact[t][s]
                nchu = -(-nex // P)          # chunks actually carrying edges
                # on-device one-hot S: S[p, c, d] = (nloc[p, c] == d)
                s_t = spool.tile([P, nchu, P], FP8, tag="s")
                co = chunk_off[0]
                nl_ap = bass.AP(
                    tensor=nloc_sb.tensor,
                    offset=nloc_sb[:, co:co + nchu].offset,
                    ap=[nloc_sb[:, co:co + nchu].ap[0],
                        nloc_sb[:, co:co + nchu].ap[1], [0, P]],
                )
                io_ap = bass.AP(
                    tensor=iota.tensor, offset=iota.offset,
                    ap=[iota.ap[0], [0, nchu], iota.ap[1]],
                )
                nc.vector.tensor_tensor(
                    out=s_t, in0=nl_ap, in1=io_ap, op=mybir.AluOpType.is_equal,
                )
                g_t = gpool.tile([P, nch, 2 * C2], FP8, tag="g")
                nc.gpsimd.dma_gather(
                    g_t, tab_d[TBOUND[t]:TBOUND[t + 1], :],
                    idx_all[:, idx_off[0]:idx_off[0] + ssz // 16],
                    num_idxs=ssz, num_idxs_reg=nex,
                    elem_size=2 * C2, transpose=False,
                    queue_num=(t * nsup + s) % KQUEUES,
                )
                idx_off[0] += ssz // 16
                chunk_off[0] += nch

                acc = acc_p.tile([P, 2 * C2], F32, tag="acc")
                npair = nchu // 2
                for ci in range(npair):
                    nc.tensor.matmul(
                        acc, lhsT=s_t[:, 2 * ci:2 * ci + 2, :],
                        rhs=g_t[:, 2 * ci:2 * ci + 2, :],
                        start=(ci == 0), stop=(ci == npair - 1 and nchu % 2 == 0),
                        perf_mode=mybir.MatmulPerfMode.DoubleRow,
                    )
                if nchu % 2 == 1:
                    nc.tensor.matmul(
                        acc, lhsT=s_t[:, nchu - 1, :], rhs=g_t[:, nchu - 1, :],
                        start=(nchu == 1), stop=True,
                    )
                if t < NSUB - 1:
                    acc_fold(t, s, acc)
                else:
                    bi = s % TB
                    if bi == 0:
                        sum_tile = p2w.tile([P, TB, 2 * C2], F32, tag="sum")
                        t_tile = p2w.tile([P, TB, C2], BF16, tag="t")
                        batch["sum"], batch["t"] = sum_tile, t_tile
                    sum_b, t_b = batch["sum"], batch["t"]
                    nc.vector.tensor_tensor(
                        out=sum_b[:, bi, :], in0=acc_sb[:, s, :],
                        in1=acc, op=mybir.AluOpType.add,
                    )
                    rec = p2w.tile([P, C2], F32, tag="rec")
                    nc.vector.reciprocal_approx_fast(rec, sum_b[:, bi, 0:C2])
                    nc.vector.tensor_tensor(
                        out=t_b[:, bi, :], in0=sum_b[:, bi, C2:2 * C2],
                        in1=rec, op=mybir.AluOpType.mult,
                    )
                    if bi == TB - 1 or s == nsup - 1:
                        nb_ = bi + 1
                        s0 = s - bi
                        tT_b = p2w.tile([P, 2 * TB, P], BF16, tag="tT")
                        nc.scalar.dma_start(
                            out=tT_b[:, 0:2 * nb_, :], in_=t_b[:, 0:nb_, :],
                            transpose=True,
                        )
                        oo = opool.tile([P, TB, C_OUT], F32)
                        for si in range(nb_):
                            o_ps = ops_p.tile([P, C_OUT], F32, tag="o")
                            for k in range(2):
                                nc.tensor.matmul(
                                    o_ps, lhsT=tT_b[:, 2 * si + k, :],
                                    rhs=wms[:, k, :],
                                    start=(k == 0), stop=(k == 1),
                                )
                            nc.vector.tensor_tensor(
                                out=oo[:, si, :], in0=o_ps, in1=bmb,
                                op=mybir.AluOpType.add,
                            )
                        nc.scalar.dma_start(
                            out=bass.AP(
                                tensor=out_d, offset=s0 * SUP * C_OUT,
                                ap=[[C_OUT, P], [SUP * C_OUT, nb_], [1, C_OUT]],
                            ),
                            in_=oo[:, 0:nb_, :],
                        )

            # ---- interleaved schedule ------------------------------------
            for t in range(NSUB):
                done = 0
                for i in range(n_strips[t]):
                    build_strip(TBOUND[t] + i * STRIP)
                    if t > 0:
                        goal = min(nsup, (nsup * (i + 1)) // n_strips[t])
                        while done < goal:
                            process_sup(t - 1, done)
                            done += 1
                if t > 0:
                    while done < nsup:
                        process_sup(t - 1, done)
                        done += 1
            # interleave the two post-build rounds (r3 remainder handled
            # above; here r4 lags r3 by the build loop) — r4 sup s needs
            # acc_sb[s] final, i.e. r3 sup s done, which the program order
            # guarantees.
            for s in range(nsup):
                process_sup(NSUB - 1, s)

    nc.compile()
    return nc


def _get_prog(nsup, sizes, exact):
    key = (nsup, tuple(map(tuple, sizes)), tuple(map(tuple, exact)),
           KQUEUES, KSCRATCH, GBUFS)
    if key not in _PROG_CACHE:
        _PROG_CACHE[key] = build_program(nsup, sizes, exact)
    return _PROG_CACHE[key]


def _wrap16(flat):
    """int16 index layout for dma_gather: [16-partition wrap, replicated x8]."""
    n = flat.shape[0]
    w = flat.reshape(n // 16, 16).T              # [16, n/16]
    return np.tile(w, (8, 1))                    # [128, n/16]


def prep_inputs(features, neighbor_idx, W1, b1, gamma, beta, run_mean, run_var,
                Ws, Wm, bm, n_cores=N_CORES):
    bf16 = ml_dtypes.bfloat16

    a = (gamma / np.sqrt(run_var + BN_EPS)).astype(np.float32)
    c = (beta - run_mean * a).astype(np.float32)
    wsp = (a[:, None] * Ws).astype(bf16)
    wmp = (a[:, None] * Wm).astype(bf16)
    bmv = (c.astype(np.float64) @ np.asarray(Wm, np.float64) + bm).astype(np.float32)

    fcm = np.zeros((P, NT), dtype=bf16)
    fcm[:, :N_NODES] = np.asarray(features, np.float32).T.astype(bf16)

    n_pc = neighbor_idx.shape[0] // n_cores
    nodes_pc = -(-n_pc // SUP) * SUP
    nsup = nodes_pc // SUP

    ni_all = np.asarray(neighbor_idx, np.int64)
    # core_edges[core][sup][sub] = (j_rel sorted, n_local)
    core_edges = []
    cnt = np.zeros((NSUB, n_cores, nsup), np.int64)
    for ci in range(n_cores):
        ni = ni_all[ci * n_pc:(ci + 1) * n_pc]
        if nodes_pc != n_pc:
            ni = np.concatenate(
                [ni, np.zeros((nodes_pc - n_pc, K_NBR), np.int64)], axis=0)
        per_sup = []
        for s in range(nsup):
            e = ni[s * SUP:(s + 1) * SUP]
            nloc = np.repeat(np.arange(SUP), K_NBR)
            j = e.reshape(-1)
            subs = []
            for t in range(NSUB):
                m = (j >= TBOUND[t]) & (j < TBOUND[t + 1])
                jt, nt_ = j[m] - TBOUND[t], nloc[m]
                o = np.argsort(jt, kind="stable")   # row-sorted for HBM locality
                subs.append((jt[o], nt_[o]))
                cnt[t, ci, s] = m.sum()
            per_sup.append(subs)
        core_edges.append(per_sup)

    sizes = [
        [int(-(-cnt[t, :, s].max() // P) * P) for s in range(nsup)]
        for t in range(NSUB)
    ]
    exact = [
        [int(cnt[t, :, s].max()) for s in range(nsup)]
        for t in range(NSUB)
    ]
    for t in range(NSUB):
        for s in range(nsup):
            assert 0 < sizes[t][s] <= GCAP, (t, s, sizes[t][s])

    tot_chunks = sum(sizes[t][s] for t in range(NSUB) for s in range(nsup)) // P
    idx_tot = tot_chunks * P

    shared = dict(
        fcm=fcm, w1=np.ascontiguousarray(W1.astype(bf16)),
        wsp=np.ascontiguousarray(wsp), wmp=np.ascontiguousarray(wmp),
        b1c=np.ascontiguousarray(b1.astype(np.float32).reshape(2, P).T),
        b1r=np.ascontiguousarray(b1.astype(bf16).reshape(1, C2)),
        bmr=bmv.reshape(1, C_OUT),
        iot=np.arange(P, dtype=np.uint8).reshape(1, P),
    )

    in_maps = []
    for ci in range(n_cores):
        idx_cols = []
        nloc_cols = []
        for t in range(NSUB):
            for s in range(nsup):
                jl, nl = core_edges[ci][s][t]
                size = sizes[t][s]
                idx = np.zeros(size, np.int16)
                idx[:len(jl)] = jl.astype(np.int16)
                idx_cols.append(idx)
                nlc = np.full(size, 255, np.uint8)
                nlc[:len(nl)] = nl.astype(np.uint8)
                nloc_cols.append(nlc)
        idx_flat = np.concatenate(idx_cols)
        assert idx_flat.shape[0] == idx_tot
        idxs = np.ascontiguousarray(_wrap16(idx_flat))
        # nloc layout: value for (chunk c, lane p) at [p, c]
        nlocs = np.ascontiguousarray(
            np.concatenate(nloc_cols).reshape(tot_chunks, P).T)
        in_maps.append(dict(shared, idxs=idxs, nlocs=nlocs))

    return in_maps, nsup, sizes, exact, n_pc


def kernel(**inputs):
    in_maps, nsup, sizes, exact, n_pc = prep_inputs(**inputs)
    nc = _get_prog(nsup, sizes, exact)
    res = run_bass_kernel_spmd(nc, in_maps, core_ids=list(range(N_CORES)))
    return np.concatenate([r["out"][:n_pc] for r in res.results], axis=0)
